# revision 6
# baseline (speedup 1.0000x reference)
"""Distributed Trainium2 kernel for the AttrClassifier masked soft-margin loss.

reference:
    scores = features @ W.T + b          # [512, 600]
    elem   = mask * (y*logsig(s) + (1-y)*logsig(-s))
           = mask * (y*s - softplus(s))  # identity: logsig(s)-logsig(-s)=s
    loss   = -mean(elem)

Sharding: the contraction dim D=25088 is split 8 ways (3136 per core), so
each core reads 1/8 of features AND 1/8 of W (~14 MB/core instead of the
~67 MB/core a batch-parallel split would need). Each core computes partial
scores.T [600, 512] in bf16, one ReduceScatter(add) combines them so core i
holds the full-precision-summed scores for classes [75i, 75i+75), and a
small fused epilogue reduces mask*(y*(s+b) - softplus(s+b)) to per-class
partial sums [75, 1]. The host sums the 8x75 partials and scales.

Host-side prep (untimed): shards are sliced and transposed so the
contraction dim lands on SBUF partitions naturally, and padded from 3136
to 3200 rows (25 uniform chunks of 128, zero rows contribute nothing).
"""

import numpy as np

B, C, D = 512, 600, 25088
NCORES = 8
DSH = D // NCORES       # 3136 contraction rows per core
KCH = 25                # 128-row contraction chunks per core (after pad)
DPAD = KCH * 128        # 3200
GRP = 5                 # chunks per DMA group / groups total
CSH = C // NCORES       # 75 classes per core after ReduceScatter
CT = 5                  # c tiles per core for matmul
CTW = C // CT           # 120 (psum partition dim, [120, 512] f32 = 1 bank)

_CACHE = {}


def _build():
    """Build + compile the SPMD Bass graph (cached; identical on all cores)."""
    if "nc" in _CACHE:
        return _CACHE["nc"]
    import concourse.bacc as bacc
    import concourse.mybir as mybir
    import concourse.tile as tile

    f32 = mybir.dt.float32
    bf16 = mybir.dt.bfloat16
    i32 = mybir.dt.int32

    nc = bacc.Bacc("TRN2", target_bir_lowering=False, debug=False,
                   num_devices=NCORES)

    ft = nc.dram_tensor("ft", [DPAD, B], f32, kind="ExternalInput")
    wt = nc.dram_tensor("wt", [DPAD, C], f32, kind="ExternalInput")
    at = nc.dram_tensor("at", [CSH, B], i32, kind="ExternalInput")
    mt = nc.dram_tensor("mt", [CSH, B], f32, kind="ExternalInput")
    bs = nc.dram_tensor("bs", [CSH, 1], f32, kind="ExternalInput")
    out = nc.dram_tensor("out", [CSH, 1], f32, kind="ExternalOutput")

    with tile.TileContext(nc) as tc:
        with (
            tc.tile_pool(name="fin", bufs=GRP) as fin,
            tc.tile_pool(name="win", bufs=GRP) as win,
            tc.tile_pool(name="sc", bufs=CT) as scp,
            tc.tile_pool(name="epi", bufs=1) as epi,
            tc.tile_pool(name="ps", bufs=1, space="PSUM") as psp,
            tc.tile_pool(name="dram", bufs=1, space="DRAM") as dram,
        ):
            # epilogue inputs early so their DMAs ride along with the big loads
            at_sb = epi.tile([CSH, B], i32, tag="at")
            mt_sb = epi.tile([CSH, B], f32, tag="mt")
            b_sb = epi.tile([CSH, 1], f32, tag="bs")
            nc.sync.dma_start(at_sb[:], at[:])
            nc.sync.dma_start(mt_sb[:], mt[:])
            nc.sync.dma_start(b_sb[:], bs[:])

            # grouped cast-DMAs: [640, X] f32 DRAM -> [128, 5*X] bf16 SBUF
            ftv = ft[:].rearrange("(g kk p) j -> g p kk j", g=GRP, kk=GRP, p=128)
            wtv = wt[:].rearrange("(g kk p) j -> g p kk j", g=GRP, kk=GRP, p=128)
            fgs, wgs = [], []
            for g in range(GRP):
                fg = fin.tile([128, GRP * B], bf16, tag="fg")
                wg = win.tile([128, GRP * C], bf16, tag="wg")
                nc.gpsimd.dma_start(
                    fg[:].rearrange("p (kk j) -> p kk j", kk=GRP), ftv[g])
                nc.gpsimd.dma_start(
                    wg[:].rearrange("p (kk j) -> p kk j", kk=GRP), wtv[g])
                fgs.append(fg)
                wgs.append(wg)

            # partial scores.T: psum[j] = wt[:, 120j:120j+120].T @ ft -> [120, 512]
            pss = [psp.tile([CTW, B], f32, tag=f"ps{j}", name=f"ps{j}")
                   for j in range(CT)]
            for k in range(KCH):
                g, kk = divmod(k, GRP)
                rhs = fgs[g][:, kk * B:(kk + 1) * B]
                for j in range(CT):
                    lhsT = wgs[g][:, kk * C + j * CTW: kk * C + (j + 1) * CTW]
                    nc.tensor.matmul(pss[j][:], lhsT, rhs,
                                     start=(k == 0), stop=(k == KCH - 1))

            # psum -> bf16 sbuf -> DRAM bounce [600, 512]
            bounce = dram.tile([C, B], bf16)
            rs_out = dram.tile([CSH, B], bf16)
            for j in range(CT):
                sc = scp.tile([CTW, B], bf16, tag="sc")
                nc.vector.tensor_copy(sc[:], pss[j][:])
                nc.sync.dma_start(bounce[j * CTW:(j + 1) * CTW, :], sc[:])

            nc.gpsimd.collective_compute(
                "ReduceScatter",
                mybir.AluOpType.add,
                replica_groups=[list(range(NCORES))],
                ins=[bounce[:].opt()],
                outs=[rs_out[:].opt()],
            )

            # epilogue on this core's class slice: rowsum_c = sum_b mask*(y*(s+b) - sp(s+b))
            s_sb = epi.tile([CSH, B], bf16, tag="s")
            nc.sync.dma_start(s_sb[:], rs_out[:])
            y = epi.tile([CSH, B], f32, tag="y")
            nc.vector.tensor_copy(y[:], at_sb[:])
            # softplus(s+b) = ln(exp(s+b) + 1); Exp and Ln share one ACT table
            ex = epi.tile([CSH, B], f32, tag="ex")
            nc.scalar.activation(ex[:], s_sb[:],
                                 mybir.ActivationFunctionType.Exp,
                                 bias=b_sb[:, :], scale=1.0)
            sp = epi.tile([CSH, B], f32, tag="sp")
            nc.scalar.activation(sp[:], ex[:],
                                 mybir.ActivationFunctionType.Ln,
                                 bias=1.0, scale=1.0)
            s2 = epi.tile([CSH, B], f32, tag="s2")
            nc.vector.tensor_scalar_add(s2[:], s_sb[:], b_sb[:, :])
            t = epi.tile([CSH, B], f32, tag="t")
            nc.vector.tensor_mul(t[:], y[:], s2[:])
            u = epi.tile([CSH, B], f32, tag="u")
            nc.vector.tensor_sub(u[:], t[:], sp[:])
            e = epi.tile([CSH, B], f32, tag="e")
            nc.vector.tensor_mul(e[:], u[:], mt_sb[:])
            rowsum = epi.tile([CSH, 1], f32, tag="rowsum")
            nc.vector.reduce_sum(out=rowsum[:], in_=e[:],
                                 axis=mybir.AxisListType.X)
            nc.sync.dma_start(out[:], rowsum[:])

    nc.compile()
    _CACHE["nc"] = nc
    return nc


def _shard(features, W, b, attr, loss_mask):
    """FULL inputs -> list of 8 per-core input maps (layout prep, untimed)."""
    features = np.ascontiguousarray(features, dtype=np.float32)
    W = np.ascontiguousarray(W, dtype=np.float32)
    b = np.ascontiguousarray(b, dtype=np.float32)
    attr = np.ascontiguousarray(attr, dtype=np.int32)
    loss_mask = np.ascontiguousarray(loss_mask, dtype=np.float32)

    attr_t = np.ascontiguousarray(attr.T)          # [600, 512]
    mask_t = np.ascontiguousarray(loss_mask.T)     # [600, 512]

    in_maps = []
    for i in range(NCORES):
        dsl = slice(i * DSH, (i + 1) * DSH)
        csl = slice(i * CSH, (i + 1) * CSH)
        ft_i = np.zeros((DPAD, B), dtype=np.float32)
        ft_i[:DSH] = features[:, dsl].T
        wt_i = np.zeros((DPAD, C), dtype=np.float32)
        wt_i[:DSH] = W[:, dsl].T
        in_maps.append({
            "ft": ft_i,
            "wt": wt_i,
            "at": np.ascontiguousarray(attr_t[csl]),
            "mt": np.ascontiguousarray(mask_t[csl]),
            "bs": np.ascontiguousarray(b[csl].reshape(CSH, 1)),
        })
    return in_maps


def _finish(results):
    """Per-core [75,1] partial sums -> full scalar loss."""
    total = 0.0
    for r in results:
        total += float(r["out"].astype(np.float64).sum())
    return np.array(-total / (B * C), dtype=np.float32)


def kernel(features, W, b, attr, loss_mask):
    from concourse.bass_utils import run_bass_kernel_spmd

    nc = _build()
    in_maps = _shard(features, W, b, attr, loss_mask)
    res = run_bass_kernel_spmd(nc, in_maps, core_ids=list(range(NCORES)))
    return _finish(res.results)
